# revision 1
# baseline (speedup 1.0000x reference)
"""Trainium2 Bass kernel for nn_Attn_block (dense transformer block).

Sharding: core i = (batch b = i//4, head-group g = i%4).  Each core computes
keys/queries/attention for its 4 heads of its batch, exchanges attention
output head-chunks for L-column chunks via two 8-rank AllToAlls, then runs
the projection + FFN + residuals on its [C, 512] column slice.

All matmuls run in float32r (full PE rate); softmax exp runs on ScalarE with
fused row-sum (accum_out); the softmax normalization is folded into the
attention-apply matmul's stationary operand (xT * (1/d)).
"""
import contextlib
import numpy as np

import concourse.bass as bass
import concourse.mybir as mybir
import concourse.tile as tile
from concourse.vector_clock import ScopedClock

# ---------------------------------------------------------------------------
# Workaround: this walrus build allows only ONE sync-wait on CTRL_NO
# (Drain/Nop) instructions; Tile's tail drain carries one wait per active
# proc.  Split the waits across single-wait nops.
# ---------------------------------------------------------------------------


def _patched_drain_and_barrier(self, tick_clock, wait_clock):
    probe = self.nc.sync.nop(nofuse=True, hint="drain_wait_split")
    wait_clock.add_sem_waits(probe.ins, ScopedClock({None: tick_clock.global_clock}))
    si = probe.ins.sync_info
    waits = list(si.on_wait) if si and si.on_wait else []
    if len(waits) > 1:
        si.on_wait = waits[:1]
        for w in waits[1:]:
            n2 = self.nc.sync.nop(nofuse=True, hint="drain_wait_split")
            si2 = n2.ins.sync_info
            if si2 is None:
                n2.ins.sync_info = mybir.SyncInfo(on_wait=[w], on_update=[])
            else:
                si2.on_wait = [w]
    self.nc.sync.drain()
    self.nc.all_engine_barrier()
    assert self.sems is not None
    popped = self.nc._tile_sem_poison_stack.pop()
    assert popped is self._sem_poison
    self.nc.clear_and_free_semaphores(list(self.sems.allocated().values()))
    self.nc.all_engine_barrier()


tile.TileContext._drain_and_barrier = _patched_drain_and_barrier


def _split_excess_waits(nc, dma_limit=1):
    """Cap per-instruction sync waits at 1 (this walrus build's limit for
    several TPB instruction structs); move excess waits onto same-engine
    NOPs inserted immediately before the instruction."""
    for bb in nc.main_func.blocks:
        insts = bb.instructions
        out = []
        for inst in insts:
            si = inst.sync_info
            waits = list(si.on_wait) if si and si.on_wait else []
            is_dma = type(inst).__name__ in ("InstDMACopy", "InstTensorLoad",
                                             "InstTensorSave")
            lim = dma_limit if is_dma else 1
            if lim is not None and len(waits) > lim:
                keep = waits[-lim:] if lim else []
                excess = waits[:-lim] if lim else waits
                eng = nc.engines[inst.engine]
                for w in excess:
                    n = eng.nop(nofuse=True, hint="wait_split")
                    # nop() appended itself to the current bb; relocate it
                    for bb2 in nc.main_func.blocks:
                        if bb2.instructions and bb2.instructions[-1] is n.ins:
                            bb2.instructions.pop()
                            break
                    n.ins.sync_info = mybir.SyncInfo(on_wait=[w], on_update=[])
                    out.append(n.ins)
                si.on_wait = keep
            out.append(inst)
        insts[:] = out


# ---------------------------------------------------------------------------

P = 128          # partitions
C = 1024         # channels
L = 2048         # sequence length
CH = 256         # channels per core (4 heads)
HD = 64          # head dim
NLB = 16         # l-blocks (L / P)
MB = 512         # matmul free-dim block
MH = 1024        # m-half for softmax tiles
N_CORES = 8
F32 = mybir.dt.float32
BF16 = mybir.dt.bfloat16


def _r(ap):
    return ap


def build_nc():
    nc = bass.Bass("TRN2", target_bir_lowering=False, debug=False,
                   num_devices=N_CORES)
    AF = mybir.ActivationFunctionType
    ALU = mybir.AluOpType

    x_d = nc.dram_tensor("x", [C, L], BF16, kind="ExternalInput")
    xT_d = nc.dram_tensor("xT", [L, CH], BF16, kind="ExternalInput")
    xsl_d = nc.dram_tensor("xsl", [C, MB], F32, kind="ExternalInput")
    kwT_d = nc.dram_tensor("kwT", [C, CH], BF16, kind="ExternalInput")
    qwT_d = nc.dram_tensor("qwT", [C, CH], BF16, kind="ExternalInput")
    pwT_d = nc.dram_tensor("pwT16", [2 * C, C], BF16, kind="ExternalInput")
    c1wT_d = nc.dram_tensor("c1wT", [C, C], BF16, kind="ExternalInput")
    c2wT_d = nc.dram_tensor("c2wT", [C, C], BF16, kind="ExternalInput")
    kb_d = nc.dram_tensor("kb2", [2, P], F32, kind="ExternalInput")
    qb_d = nc.dram_tensor("qb2", [2, P], F32, kind="ExternalInput")
    pb_d = nc.dram_tensor("pb8", [8, P], F32, kind="ExternalInput")
    c1b_d = nc.dram_tensor("c1b8", [8, P], F32, kind="ExternalInput")
    c2b_d = nc.dram_tensor("c2b8", [8, P], F32, kind="ExternalInput")
    out_d = nc.dram_tensor("out", [C, MB], F32, kind="ExternalOutput")

    with tile.TileContext(nc) as tc, contextlib.ExitStack() as ctx:
        dram = ctx.enter_context(tc.tile_pool(name="dram", bufs=1, space="DRAM"))
        a2a_in = [dram.tile([N_CORES, P, MB], BF16, name=f"a2a_in{p}", tag=f"ai{p}")
                  for p in range(2)]
        a2a_out = [dram.tile([N_CORES, P, MB], BF16, name=f"a2a_out{p}", tag=f"ao{p}")
                   for p in range(2)]

        # --- persistent pools ------------------------------------------------
        biasp = ctx.enter_context(tc.tile_pool(name="biasp", bufs=1))
        kb_sb = biasp.tile([P, 2], F32, name="kb_sb", tag="kb")
        qb_sb = biasp.tile([P, 2], F32, name="qb_sb", tag="qb")
        pb_sb = biasp.tile([P, 8], F32, name="pb_sb", tag="pb")
        c1b_sb = biasp.tile([P, 8], F32, name="c1b_sb", tag="c1b")
        c2b_sb = biasp.tile([P, 8], F32, name="c2b_sb", tag="c2b")
        warmp = ctx.enter_context(tc.tile_pool(name="warmp", bufs=1))
        warm_t = warmp.tile([P, 1], F32, name="warm_t", tag="warm")
        nc.any.memset(warm_t[:], 0.0)
        nc.scalar.activation(warm_t[:], warm_t[:], AF.Exp)

        xslp = ctx.enter_context(tc.tile_pool(name="xslp", bufs=1))
        xsl_sb = [xslp.tile([P, MB], F32, name=f"xsl{o}", tag=f"xsl{o}")
                  for o in range(8)]

        # keys/queries/xT live through phases A+B only
        phb = contextlib.ExitStack()
        kqp = phb.enter_context(tc.tile_pool(name="kqp", bufs=1))
        keys_sb = [kqp.tile([P, L], BF16, name=f"keys{p}", tag=f"k{p}")
                   for p in range(2)]
        qrys_sb = [kqp.tile([P, L], BF16, name=f"qrys{p}", tag=f"q{p}")
                   for p in range(2)]
        xTp = phb.enter_context(tc.tile_pool(name="xTp", bufs=1))
        xT_sb = [xTp.tile([P, CH], BF16, name=f"xT{j}", tag=f"xT{j}")
                 for j in range(NLB)]

        # --- phase A: k/q convolutions ---------------------------------------
        with tc.tile_pool(name="xp", bufs=1) as xp, \
             tc.tile_pool(name="kqwp", bufs=1) as kqwp, \
             tc.tile_pool(name="convps", bufs=4, space="PSUM") as convps:
            x_sb = [xp.tile([P, L], BF16, name=f"x{t}", tag=f"x{t}")
                    for t in range(8)]
            kwT_sb = [kqwp.tile([P, CH], BF16, name=f"kwT{t}", tag=f"kw{t}")
                      for t in range(8)]
            qwT_sb = [kqwp.tile([P, CH], BF16, name=f"qwT{t}", tag=f"qw{t}")
                      for t in range(8)]
            for t in range(8):
                nc.sync.dma_start(qwT_sb[t][:], qwT_d[P * t:P * (t + 1), :])
                nc.sync.dma_start(kwT_sb[t][:], kwT_d[P * t:P * (t + 1), :])
                eng = nc.gpsimd if t % 2 else nc.sync
                eng.dma_start(x_sb[t][:], x_d[P * t:P * (t + 1), :])
            nc.sync.dma_start(kb_sb[:], kb_d.rearrange("t p -> p t"))
            nc.sync.dma_start(qb_sb[:], qb_d.rearrange("t p -> p t"))
            nc.sync.dma_start(pb_sb[:], pb_d.rearrange("t p -> p t"))
            nc.sync.dma_start(c1b_sb[:], c1b_d.rearrange("t p -> p t"))
            nc.sync.dma_start(c2b_sb[:], c2b_d.rearrange("t p -> p t"))
            for j in range(NLB):
                nc.sync.dma_start(xT_sb[j][:], xT_d[P * j:P * (j + 1), :])
            for o in range(8):
                nc.sync.dma_start(xsl_sb[o][:], xsl_d[P * o:P * (o + 1), :])

            for dst, w_sb, b_sb in ((qrys_sb, qwT_sb, qb_sb),
                                    (keys_sb, kwT_sb, kb_sb)):
                for m in range(2):          # chunk-local 128-channel tile
                    for n in range(4):      # 512-wide l blocks
                        ps = convps.tile([P, MB], F32, name="convps", tag="cps")
                        for t in range(8):
                            nc.tensor.matmul(
                                ps[:],
                                _r(w_sb[t][:, P * m:P * (m + 1)]),
                                _r(x_sb[t][:, MB * n:MB * (n + 1)]),
                                start=(t == 0), stop=(t == 7),
                            )
                        nc.vector.tensor_scalar_add(
                            dst[m][:, MB * n:MB * (n + 1)], ps[:],
                            b_sb[:, m:m + 1])

        # --- phase B: attention per head pair --------------------------------
        with tc.tile_pool(name="scoresps", bufs=2, space="PSUM") as scoresps, \
             tc.tile_pool(name="applyps", bufs=1, space="PSUM") as applyps, \
             tc.tile_pool(name="ep", bufs=12) as ep, \
             tc.tile_pool(name="dp", bufs=16) as dp, \
             tc.tile_pool(name="xs2p", bufs=3) as xs2p, \
             tc.tile_pool(name="attnp", bufs=1) as attnp:
            for p in range(2):
                apl = applyps.tile([P, L], F32, name="apl", tag="apl")

                def emit_apply(st, mhs):
                    j, e_t, xs2 = st
                    for mh in mhs:
                        for k in range(2):
                            for h in range(2):
                                col = MH * mh + MB * k
                                nc.tensor.matmul(
                                    apl[HD * h:HD * (h + 1), col:col + MB],
                                    _r(xs2[:, HD * h:HD * (h + 1)]),
                                    _r(e_t[(h, mh)][:, MB * k:MB * (k + 1)]),
                                    start=(j == 0), stop=(j == NLB - 1),
                                    tile_position=(0, HD * h),
                                )

                pending = None
                for j in range(NLB):
                    e_t = {}
                    dacc = {}
                    for mh in range(2):  # m half
                        sc = {h: scoresps.tile([P, MH], F32, name="sc", tag="sc")
                              for h in range(2)}
                        # interleave head A/B so the row-group pairs overlap
                        # on the PE array
                        for k in range(2):
                            for h in range(2):
                                hp = HD * h
                                nc.tensor.matmul(
                                    sc[h][:, MB * k:MB * (k + 1)],
                                    _r(keys_sb[p][hp:hp + HD, P * j:P * (j + 1)]),
                                    _r(qrys_sb[p][hp:hp + HD,
                                                  MH * mh + MB * k:
                                                  MH * mh + MB * (k + 1)]),
                                    start=True, stop=True,
                                )
                        for h in range(2):
                            e = ep.tile([P, MH], BF16, name="e", tag="e")
                            d = dp.tile([P, 1], F32, name="d", tag="d")
                            nc.scalar.activation(e[:], sc[h][:], AF.Exp,
                                                 accum_out=d[:])
                            e_t[(h, mh)] = e
                            dacc[(h, mh)] = d
                    xs2 = xs2p.tile([P, P], BF16, name="xs2", tag="xs2")
                    for h in range(2):
                        ds = dp.tile([P, 1], F32, name="ds", tag="ds")
                        nc.vector.tensor_add(ds[:], dacc[(h, 0)][:],
                                             dacc[(h, 1)][:])
                        rc = dp.tile([P, 1], F32, name="rc", tag="rc")
                        nc.vector.reciprocal(rc[:], ds[:])
                        nc.vector.tensor_scalar(
                            xs2[:, HD * h:HD * (h + 1)],
                            xT_sb[j][:, P * p + HD * h:P * p + HD * (h + 1)],
                            rc[:], None, op0=ALU.mult)
                    if pending is not None:
                        emit_apply(pending, (0, 1))
                    pending = (j, e_t, xs2)
                emit_apply(pending, (0, 1))
                # stage p exchange: duplicate halves so the shard pattern is
                # core-independent; receivers mask wrong-batch slots via the
                # zero rows of pwT16.
                attn_sb = attnp.tile([P, L], BF16, name="attn_sb", tag="at")
                nc.vector.tensor_copy(attn_sb[:], apl[:])
                a3 = attn_sb[:].rearrange("p (j m) -> p j m", j=4)
                nc.sync.dma_start(a2a_in[p][0:4].rearrange("j p m -> p j m"), a3)
                nc.sync.dma_start(a2a_in[p][4:8].rearrange("j p m -> p j m"), a3)
                nc.gpsimd.collective_compute(
                    "AllToAll", ALU.bypass,
                    replica_groups=[list(range(N_CORES))],
                    ins=[a2a_in[p][:]],
                    outs=[a2a_out[p][:]],
                )

        # --- phase C: projection + FFN on the local column slice -------------
        phb.close()  # release keys/queries/xT SBUF
        with tc.tile_pool(name="yp", bufs=1) as yp, \
             tc.tile_pool(name="gp", bufs=1) as gp, \
             tc.tile_pool(name="pwsp", bufs=4) as pwsp, \
             tc.tile_pool(name="wp2", bufs=1) as wp2, \
             tc.tile_pool(name="ph2ps", bufs=1, space="PSUM") as ph2ps:
            g_sb = [gp.tile([P, MB], BF16, name=f"g{t}", tag=f"g{t}")
                    for t in range(16)]
            for t in range(16):
                nc.sync.dma_start(g_sb[t][:], a2a_out[t // 8][t % 8])

            c1wT_sb = [wp2.tile([P, C], BF16, name=f"c1wT{t}", tag=f"c1w{t}")
                       for t in range(8)]
            c2wT_sb = [wp2.tile([P, C], BF16, name=f"c2wT{t}", tag=f"c2w{t}")
                       for t in range(8)]

            y_sb = [yp.tile([P, MB], F32, name=f"y{o}", tag=f"y{o}")
                    for o in range(8)]
            yb_sb = [yp.tile([P, MB], BF16, name=f"yb{o}", tag=f"yb{o}")
                     for o in range(8)]
            r_sb = [yp.tile([P, MB], BF16, name=f"r{o}", tag=f"r{o}")
                    for o in range(8)]

            # pw projection, pwT streamed from DRAM: all 8 o-psums live,
            # loop over the 16 k-tiles in the outer loop.
            pw_ps = [ph2ps.tile([P, MB], F32, name=f"pwps{o}", tag=f"p2{o}")
                     for o in range(8)]
            for t in range(16):
                pwt = pwsp.tile([P, C], BF16, name="pwt", tag="pwt")
                nc.sync.dma_start(pwt[:], pwT_d[P * t:P * (t + 1), :])
                for o in range(8):
                    nc.tensor.matmul(
                        pw_ps[o][:], _r(pwt[:, P * o:P * (o + 1)]),
                        _r(g_sb[t][:]), start=(t == 0), stop=(t == 15))
            for t in range(8):
                nc.sync.dma_start(c1wT_sb[t][:], c1wT_d[P * t:P * (t + 1), :])
                nc.sync.dma_start(c2wT_sb[t][:], c2wT_d[P * t:P * (t + 1), :])
            for o in range(8):
                # y = pw_out + pb + x_slice
                nc.vector.scalar_tensor_tensor(
                    y_sb[o][:], pw_ps[o][:], pb_sb[:, o:o + 1], xsl_sb[o][:],
                    op0=ALU.add, op1=ALU.add)
                nc.vector.tensor_copy(yb_sb[o][:], y_sb[o][:])

            yx_sb = [yp.tile([P, MB], F32, name=f"yx{o}", tag=f"yx{o}")
                     for o in range(8)]
            for o in range(8):
                nc.vector.tensor_add(yx_sb[o][:], y_sb[o][:], xsl_sb[o][:])

            # c1 + relu
            for o in range(8):
                ps = ph2ps.tile([P, MB], F32, name="c1ps", tag=f"p2{o}")
                for t in range(8):
                    nc.tensor.matmul(
                        ps[:], _r(c1wT_sb[t][:, P * o:P * (o + 1)]),
                        _r(yb_sb[t][:]), start=(t == 0), stop=(t == 7))
                nc.scalar.activation(r_sb[o][:], ps[:], AF.Relu,
                                     bias=c1b_sb[:, o:o + 1])

            # c2 + residuals: out = c2conv + c2b + y + x_slice
            for o in range(8):
                ps = ph2ps.tile([P, MB], F32, name="c2ps", tag=f"p2{o}")
                for t in range(8):
                    nc.tensor.matmul(
                        ps[:], _r(c2wT_sb[t][:, P * o:P * (o + 1)]),
                        _r(r_sb[t][:]), start=(t == 0), stop=(t == 7))
                nc.vector.scalar_tensor_tensor(
                    y_sb[o][:], ps[:], c2b_sb[:, o:o + 1], yx_sb[o][:],
                    op0=ALU.add, op1=ALU.add)
                nc.sync.dma_start(out_d[P * o:P * (o + 1), :], y_sb[o][:])

    _split_excess_waits(nc)
    return nc


_NC = None


def _get_nc():
    global _NC
    if _NC is None:
        _NC = build_nc()
    return _NC


def _prep_inputs(x, kw, kb, qw, qb, pw, pb, c1w, c1b, c2w, c2b):
    """Build the 8 per-core input maps."""
    import ml_dtypes
    f = np.float32
    bf = ml_dtypes.bfloat16
    cc = lambda a: np.ascontiguousarray(a, dtype=f)
    cb = lambda a: np.ascontiguousarray(np.asarray(a, dtype=f), dtype=bf)
    kwT = kw.T / np.float32(L / 2.0)      # fold softmax temperature
    kbs = kb / np.float32(L / 2.0)
    qwT, pwT, c1wT, c2wT = qw.T, pw.T, c1w.T, c2w.T

    in_maps = []
    for i in range(N_CORES):
        b, g = divmod(i, 4)
        ch0 = CH * g
        # pwT16: 16 x 128 row blocks; slot t = (stage p = t//8, src rank s = t%8)
        # rows = pwT[channels of src s's pair p]; zero for wrong-batch sources.
        pwT16 = np.zeros((2 * C, C), dtype=bf)
        for t in range(16):
            p_st, s = divmod(t, 8)
            if s // 4 == b:
                src_g = s % 4
                r0 = CH * src_g + P * p_st
                pwT16[P * t:P * (t + 1), :] = pwT[r0:r0 + P, :].astype(bf)
        in_maps.append({
            "x": cb(x[b]),
            "xT": cb(x[b].T[:, ch0:ch0 + CH]),
            "xsl": cc(x[b][:, MB * g:MB * (g + 1)]),
            "kwT": cb(kwT[:, ch0:ch0 + CH]),
            "qwT": cb(qwT[:, ch0:ch0 + CH]),
            "pwT16": pwT16,
            "c1wT": cb(c1wT),
            "c2wT": cb(c2wT),
            "kb2": cc(kbs[ch0:ch0 + CH].reshape(2, P)),
            "qb2": cc(qb[ch0:ch0 + CH].reshape(2, P)),
            "pb8": cc(pb.reshape(8, P)),
            "c1b8": cc(c1b.reshape(8, P)),
            "c2b8": cc(c2b.reshape(8, P)),
        })
    return in_maps


def run(inputs, trace=False, **kw):
    from concourse.bass_utils import run_bass_kernel_spmd
    nc = _get_nc()
    in_maps = _prep_inputs(**inputs)
    res = run_bass_kernel_spmd(nc, in_maps, list(range(N_CORES)),
                               trace=trace, **kw)
    out = np.empty((2, C, L), dtype=np.float32)
    for i in range(N_CORES):
        b, g = divmod(i, 4)
        out[b][:, MB * g:MB * (g + 1)] = res.results[i]["out"]
    return out, res


def kernel(**inputs) -> np.ndarray:
    out, _ = run(inputs)
    return out

